# revision 18
# baseline (speedup 1.0000x reference)
"""LSTM encoder (embedding lookup + single-layer LSTM with length masking)
on 8 Trainium2 NeuronCores, data-parallel over the batch dimension.

Layout strategy: everything in the recurrence is TRANSPOSED — hidden dim on
SBUF partitions, batch on the free dim — so per-step elementwise work runs on
[128, 32] tiles (fast) instead of [8, 2048] (slow).  Per timestep the gate
pre-activations z.T = Wh.T @ h.T + xproj.T are built in PSUM from 16 identity
matmuls (injecting xproj) plus 64 bf16 LDWEIGHTS+MATMUL pairs (N=8).
xproj.T = Wi.T @ x.T is computed on-device per 64-step block: indirect-DMA
embedding gather -> PE transposes -> GEMM with Wi stationary.
"""

import os
import sys

sys.path.insert(0, "/opt/trn_rl_repo")

import numpy as np
import ml_dtypes

import concourse.bass as bass
import concourse.tile as tile
from concourse import bacc, mybir
from concourse.bass_utils import run_bass_kernel_spmd
from concourse.masks import make_identity

F32 = mybir.dt.float32
BF16 = mybir.dt.bfloat16
I32 = mybir.dt.int32

V, E, H, B, T = 32000, 512, 512, 64, 512
NCORES = 8
BS = B // NCORES          # 8 sequences per core
TB = 64                   # timesteps per x_proj block
G4 = 4 * H                # 2048 gate columns
NMC = G4 // 128           # 16 column tiles of Wh/Wi
NKC = E // 128            # 4 contraction chunks

# PSUM gate-region layout: free offset of each mc tile inside pz [128, 1536].
# bank0: g-gate (mc 8..11) at 0..31 | bank1: i (mc 0..3) at 512..543,
# f (mc 4..7) at 544..575 | bank2: o (mc 12..15) at 1024..1055
def _mc_off(mc):
    if 8 <= mc < 12:            # g
        return 0 + (mc - 8) * 8
    if 0 <= mc < 4:             # i
        return 512 + mc * 8
    if 4 <= mc < 8:             # f
        return 544 + (mc - 4) * 8
    return 1024 + (mc - 12) * 8  # o


def build_nc(t_steps=T):
    assert t_steps % 8 == 0
    nblk = (t_steps + TB - 1) // TB
    ng8 = t_steps // 8

    nc = bacc.Bacc("TRN2", target_bir_lowering=False, debug=False)

    emb = nc.dram_tensor("emb", [V, E], F32, kind="ExternalInput").ap()
    idxd = nc.dram_tensor("idx", [nblk, 128, 4], I32, kind="ExternalInput").ap()
    whd = nc.dram_tensor("wh", [128, NKC * G4], BF16, kind="ExternalInput").ap()
    wid = nc.dram_tensor("wi", [128, NKC * G4], BF16, kind="ExternalInput").ap()
    lend = nc.dram_tensor("lent", [128, 32], F32, kind="ExternalInput").ap()
    biasd = nc.dram_tensor("bias", [128, NMC], F32, kind="ExternalInput").ap()

    outd = nc.dram_tensor("out", [ng8, 128, 256], F32, kind="ExternalOutput").ap()
    debug_xp = os.environ.get("LSTM_DEBUG_XP", "0") == "1"
    if debug_xp:
        xpdbg = nc.dram_tensor("xpdbg", [128, NMC * 512], BF16, kind="ExternalOutput").ap()
    debug_s0 = os.environ.get("LSTM_DEBUG_STEP0", "0") == "1"
    if debug_s0:
        s0dbg = nc.dram_tensor("s0dbg", [6, 128, 64], F32, kind="ExternalOutput").ap()
        pzdbg = nc.dram_tensor("pzdbg", [128, 1536], F32, kind="ExternalOutput").ap()
    fcd = nc.dram_tensor("fc", [128, 32], F32, kind="ExternalOutput").ap()
    fhd = nc.dram_tensor("fh", [128, 32], F32, kind="ExternalOutput").ap()

    with tile.TileContext(nc) as tc:
        ctxs = []
        const = tc.alloc_tile_pool(name="const", bufs=1)
        psg = tc.alloc_tile_pool(name="psg", bufs=2, space="PSUM")
        psaux = tc.alloc_tile_pool(name="psaux", bufs=2, space="PSUM")
        idxp = tc.alloc_tile_pool(name="idxp", bufs=2)
        xrawp = tc.alloc_tile_pool(name="xrawp", bufs=3)
        xtp = tc.alloc_tile_pool(name="xtp", bufs=2)
        xpp = tc.alloc_tile_pool(name="xpp", bufs=2)
        ewp = tc.alloc_tile_pool(name="ewp", bufs=2)
        stagep = tc.alloc_tile_pool(name="stagep", bufs=3)

        # ---- persistent tiles ----
        wh_sb = const.tile([128, NKC * G4], BF16)
        nc.sync.dma_start(wh_sb[:], whd)
        wi_sb = const.tile([128, NKC * G4], BF16)
        nc.sync.dma_start(wi_sb[:], wid)
        len_sb = const.tile([128, 32], F32)
        nc.sync.dma_start(len_sb[:], lend)
        bias_sb = const.tile([128, NMC], F32)
        nc.sync.dma_start(bias_sb[:], biasd)

        ident = const.tile([128, 128], F32)
        make_identity(nc, ident[:])
        ident_bf = const.tile([128, 128], BF16)
        nc.vector.tensor_copy(ident_bf[:], ident[:])

        c_car = const.tile([128, 32], F32)
        nc.vector.memset(c_car[:], 0.0)
        # h_bf: masked h carry (feeds outputs/final state only).  The matmul
        # rhs uses the UNMASKED h (finished sequences' gate columns are
        # discarded anyway), keeping copy_predicated off the critical path.
        h_bf = const.tile([128, 32], BF16)
        nc.vector.memset(h_bf[:], 0.0)
        h0 = const.tile([128, 32], BF16)
        nc.vector.memset(h0[:], 0.0)

        xproj_blocks = {}

        def make_xproj(blk):
            """Gather + transpose + GEMM for one 64-step block."""
            idx_t = idxp.tile([128, 4], I32, name=f"idx_{blk}", tag="idx")
            nc.sync.dma_start(idx_t[:], idxd[blk])
            xt_sb = xtp.tile([128, NKC * 512], BF16, name=f"xt_{blk}", tag="xt")
            for i in range(4):
                xr = xrawp.tile([128, E], F32, name=f"xr_{blk}_{i}", tag="xr")
                nc.gpsimd.indirect_dma_start(
                    out=xr[:],
                    out_offset=None,
                    in_=emb,
                    in_offset=bass.IndirectOffsetOnAxis(ap=idx_t[:, i : i + 1], axis=0),
                )
                pt = psaux.tile([128, 512], F32, name=f"pt_{blk}_{i}", tag="aux")
                for e in range(4):
                    nc.tensor.transpose(
                        pt[:, e * 128 : (e + 1) * 128],
                        xr[:, e * 128 : (e + 1) * 128],
                        ident[:],
                    )
                # strided copy: psum (e, tok128) -> xt_sb[(e)*512 + i*128 ..]
                src = pt[:].rearrange("p (e q) -> p e q", e=4)
                dst = xt_sb[:].rearrange("p (e w q) -> p e w q", e=4, w=4)[:, :, i, :]
                nc.vector.tensor_copy(dst, src)
            xp_sb = xpp.tile([128, NMC * 512], BF16, name=f"xp_{blk}", tag="xp")
            for mc in range(NMC):
                pg = psaux.tile([128, 512], F32, name=f"pg_{blk}_{mc}", tag="aux")
                for kc in range(NKC):
                    nc.tensor.matmul(
                        pg[:],
                        wi_sb[:, kc * G4 + mc * 128 : kc * G4 + (mc + 1) * 128],
                        xt_sb[:, kc * 512 : (kc + 1) * 512],
                        start=(kc == 0),
                        stop=(kc == NKC - 1),
                    )
                nc.vector.tensor_scalar_add(
                    xp_sb[:, mc * 512 : (mc + 1) * 512], pg[:], bias_sb[:, mc : mc + 1]
                )
            return xp_sb

        # MM issue order: g first, then i, f, o (so tanh(g) can start early)
        mc_order = [8, 9, 10, 11, 0, 1, 2, 3, 4, 5, 6, 7, 12, 13, 14, 15]

        stage = None
        h_prev = h0
        xp_next = make_xproj(0)
        if debug_xp:
            nc.sync.dma_start(xpdbg, xp_next[:])
        for t in range(t_steps):
            blk, t_sub = t // TB, t % TB
            if t_sub == 0:
                xp_sb = xp_next
            # prefetch next block's xproj early so its gather DMA + PE work
            # pipeline behind this block's recurrence steps
            if t_sub == 8 and blk + 1 < nblk:
                xp_next = make_xproj(blk + 1)
            if t % 8 == 0:
                stage = stagep.tile([128, 256], F32, name=f"stage_{t // 8}", tag="st")

            pz = psg.tile([128, 1536], F32, name=f"pz_{t}", tag="pz")
            # inject xproj via identity matmuls, one per gate (N=32, strided
            # rhs over the 4 column-chunks).  start=True clears has_written
            # for the WHOLE bank, so only the first idMM per bank may set it;
            # later idMMs (start=False) overwrite their never-written region.
            xp4 = xp_sb[:].rearrange("p (g c t b) -> p g c t b", g=4, c=4, b=BS)
            for g, goff, st in ((2, 0, True), (0, 512, True), (1, 544, False),
                                (3, 1024, True)):
                nc.tensor.matmul(
                    pz[:, goff : goff + 32],
                    ident_bf[:],
                    xp4[:, g, :, t_sub, :],
                    start=st,
                    stop=False,
                )
            # recurrent part: z += Wh.T @ h
            nsp = int(os.environ.get("LSTM_COLSPLIT", "0"))
            for mc in mc_order:
                off = _mc_off(mc)
                for kc in range(NKC):
                    base = kc * G4 + mc * 128
                    if nsp:
                        w = 128 // nsp
                        for j in range(nsp):
                            nc.tensor.matmul(
                                pz[j * w : (j + 1) * w, off : off + 8],
                                wh_sb[:, base + j * w : base + (j + 1) * w],
                                h_prev[:, kc * 8 : (kc + 1) * 8],
                                start=False,
                                stop=(kc == NKC - 1),
                                tile_position=(0, j * w),
                            )
                    else:
                        nc.tensor.matmul(
                            pz[:, off : off + 8],
                            wh_sb[:, base : base + 128],
                            h_prev[:, kc * 8 : (kc + 1) * 8],
                            start=False,
                            stop=(kc == NKC - 1),
                        )

            # mask for this step (independent of gates; scheduler can hoist)
            msk = ewp.tile([128, 32], mybir.dt.uint8, name=f"msk_{t}", tag="msk")
            nc.vector.tensor_scalar(
                msk[:], len_sb[:], float(t), None, op0=mybir.AluOpType.is_gt
            )

            tg = ewp.tile([128, 32], F32, name=f"tg_{t}", tag="tg")
            nc.scalar.activation(tg[:], pz[:, 0:32], mybir.ActivationFunctionType.Tanh)
            sif = ewp.tile([128, 64], F32, name=f"sif_{t}", tag="sif")
            nc.scalar.activation(
                sif[:], pz[:, 512:576], mybir.ActivationFunctionType.Sigmoid
            )
            so = ewp.tile([128, 32], F32, name=f"so_{t}", tag="so")
            nc.scalar.activation(
                so[:], pz[:, 1024:1056], mybir.ActivationFunctionType.Sigmoid
            )

            t1 = ewp.tile([128, 32], F32, name=f"t1_{t}", tag="t1")
            nc.vector.tensor_mul(t1[:], sif[:, 32:64], c_car[:])
            t2 = ewp.tile([128, 32], F32, name=f"t2_{t}", tag="t2")
            nc.vector.tensor_mul(t2[:], sif[:, 0:32], tg[:])
            cn = ewp.tile([128, 32], F32, name=f"cn_{t}", tag="cn")
            nc.vector.tensor_add(cn[:], t1[:], t2[:])
            tc_t = ewp.tile([128, 32], F32, name=f"tc_{t}", tag="tc")
            nc.scalar.activation(
                tc_t[:], cn[:], mybir.ActivationFunctionType.Tanh
            )
            nc.vector.copy_predicated(c_car[:], msk[:], cn[:])

            # h path stays bf16: shorter critical tail into the next step's
            # matmuls; outputs take the bf16-rounded h (within error budget).
            hn = ewp.tile([128, 32], BF16, name=f"hn_{t}", tag="hn")
            nc.vector.tensor_mul(hn[:], so[:], tc_t[:])
            h_prev = hn
            nc.vector.copy_predicated(h_bf[:], msk[:], hn[:])
            if debug_s0 and t == 0:
                pzsb = ewp.tile([128, 1536], F32, name="pzsb_dbg", tag="pzdbg", bufs=1)
                nc.vector.tensor_copy(pzsb[:], pz[:])
                nc.sync.dma_start(pzdbg, pzsb[:])
                for di, dt_ in enumerate([tg, so, t1, t2, cn, hn]):
                    nc.sync.dma_start(s0dbg[di][:, 0:32], dt_[:])
                nc.sync.dma_start(s0dbg[1][:, 32:64], sif[:, 0:32])
            s8 = t % 8
            nc.scalar.copy(stage[:, s8 * 32 : (s8 + 1) * 32], h_bf[:])
            if s8 == 7:
                nc.sync.dma_start(outd[t // 8], stage[:])

        nc.sync.dma_start(fcd, c_car[:])
        h_f32 = ewp.tile([128, 32], F32, name="h_f32_fin", tag="hf")
        nc.scalar.copy(h_f32[:], h_bf[:])
        nc.sync.dma_start(fhd, h_f32[:])

        for p in (stagep, ewp, xpp, xtp, xrawp, idxp, psaux, psg, const):
            p.release()

    nc.compile()
    return nc


_NC_CACHE = {}


def _get_nc(t_steps):
    if t_steps not in _NC_CACHE:
        _NC_CACHE[t_steps] = build_nc(t_steps)
    return _NC_CACHE[t_steps]


def _prep_weights(Wi, Wh, b):
    """[E, 4H] f32 -> [128, 4*4H] bf16 stationary layout (kc chunks side by side)."""
    def prep(w):
        w4 = w.reshape(NKC, 128, G4).transpose(1, 0, 2).reshape(128, NKC * G4)
        return np.ascontiguousarray(w4).astype(ml_dtypes.bfloat16)

    bias = np.ascontiguousarray(b.reshape(NMC, 128).T).astype(np.float32)
    return prep(Wi), prep(Wh), bias


def _build_in_maps(inp, t_steps):
    inputs = np.asarray(inp["inputs"], dtype=np.int32)
    lengths = np.asarray(inp["lengths"], dtype=np.int32)
    embedding = np.ascontiguousarray(np.asarray(inp["embedding"], dtype=np.float32))
    Wi = np.asarray(inp["Wi"], dtype=np.float32)
    Wh = np.asarray(inp["Wh"], dtype=np.float32)
    b = np.asarray(inp["b"], dtype=np.float32)

    wi_p, wh_p, bias_p = _prep_weights(Wi, Wh, b)

    nblk = (t_steps + TB - 1) // TB
    in_maps = []
    for k in range(NCORES):
        sh = slice(k * BS, (k + 1) * BS)
        toks = inputs[sh, :t_steps]  # [BS, t_steps]
        if toks.shape[1] < nblk * TB:  # dev mode: pad to full gather blocks
            toks = np.pad(toks, ((0, 0), (0, nblk * TB - toks.shape[1])))
        # idx[blk, p, i] = token at (t = blk*64 + (i*128+p)//8, b = (i*128+p)%8)
        jj = np.arange(4)[None, :] * 128 + np.arange(128)[:, None]  # [p, i]
        idx = np.zeros((nblk, 128, 4), dtype=np.int32)
        for blk in range(nblk):
            tt = blk * TB + jj // BS
            bb = jj % BS
            idx[blk] = toks[bb, tt]
        lent = np.broadcast_to(
            np.tile(lengths[sh].astype(np.float32), NKC)[None, :], (128, 32)
        ).copy()
        in_maps.append(
            {
                "emb": embedding,
                "idx": idx,
                "wh": wh_p,
                "wi": wi_p,
                "lent": lent,
                "bias": bias_p,
            }
        )
    return in_maps


def kernel(inputs, lengths, embedding, Wi, Wh, b, t_steps=None):
    t_steps = t_steps or int(os.environ.get("LSTM_T_STEPS", T))
    nc = _get_nc(t_steps)
    in_maps = _build_in_maps(
        {"inputs": inputs, "lengths": lengths, "embedding": embedding,
         "Wi": Wi, "Wh": Wh, "b": b},
        t_steps,
    )
    res = run_bass_kernel_spmd(nc, in_maps, core_ids=list(range(NCORES)))

    outputs = np.zeros((B, T, H), dtype=np.float32)
    c_f = np.zeros((B, H), dtype=np.float32)
    h_f = np.zeros((B, H), dtype=np.float32)
    for k in range(NCORES):
        r = res.results[k]
        # out [ng8, 128, 256]: [g8, p, (s, c, b)] -> [b, t, c*128 + p]
        o = r["out"].reshape(t_steps // 8, 128, 8, 4, BS)
        o = o.transpose(4, 0, 2, 3, 1).reshape(BS, t_steps, H)
        outputs[k * BS : (k + 1) * BS, :t_steps] = o
        # fc/fh [128, 32] = [p, (c, b)] -> [b, c*128 + p]
        fc = r["fc"].reshape(128, NKC, BS).transpose(2, 1, 0).reshape(BS, H)
        fh = r["fh"].reshape(128, NKC, BS).transpose(2, 1, 0).reshape(BS, H)
        c_f[k * BS : (k + 1) * BS] = fc
        h_f[k * BS : (k + 1) * BS] = fh
    if t_steps < T:
        # dev mode: outputs beyond t_steps replicate the frozen value like the
        # reference would only if lengths <= t_steps; leave zeros (test.py
        # compares only the first t_steps in this mode)
        pass
    return outputs, (c_f, h_f)


# revision 19
# speedup vs baseline: 1.3407x; 1.3407x over previous
"""LSTM encoder (embedding lookup + single-layer LSTM with length masking)
on 8 Trainium2 NeuronCores, data-parallel over the batch dimension.

Layout strategy: everything in the recurrence is TRANSPOSED — hidden dim on
SBUF partitions, batch on the free dim — so per-step elementwise work runs on
[128, 32] tiles (fast) instead of [8, 2048] (slow).  Per timestep the gate
pre-activations z.T = Wh.T @ h.T + xproj.T are built in PSUM from 16 identity
matmuls (injecting xproj) plus 64 bf16 LDWEIGHTS+MATMUL pairs (N=8).
xproj.T = Wi.T @ x.T is computed on-device per 64-step block: indirect-DMA
embedding gather -> PE transposes -> GEMM with Wi stationary.
"""

import os
import sys

sys.path.insert(0, "/opt/trn_rl_repo")

import numpy as np
import ml_dtypes

import concourse.bass as bass
import concourse.tile as tile
from concourse import bacc, mybir
from concourse.bass_utils import run_bass_kernel_spmd
from concourse.masks import make_identity

F32 = mybir.dt.float32
BF16 = mybir.dt.bfloat16
I32 = mybir.dt.int32

V, E, H, B, T = 32000, 512, 512, 64, 512
NCORES = 8
BS = B // NCORES          # 8 sequences per core
TB = 64                   # timesteps per x_proj block
G4 = 4 * H                # 2048 gate columns
NMC = G4 // 128           # 16 column tiles of Wh/Wi
NKC = E // 128            # 4 contraction chunks

# PSUM gate-region layout: free offset of each mc tile inside pz [128, 1536].
# bank0: g-gate (mc 8..11) at 0..31 | bank1: i (mc 0..3) at 512..543,
# f (mc 4..7) at 544..575 | bank2: o (mc 12..15) at 1024..1055
def _mc_off(mc):
    if 8 <= mc < 12:            # g
        return 0 + (mc - 8) * 8
    if 0 <= mc < 4:             # i
        return 512 + mc * 8
    if 4 <= mc < 8:             # f
        return 544 + (mc - 4) * 8
    return 1024 + (mc - 12) * 8  # o


def build_nc(t_steps=T):
    assert t_steps % 8 == 0
    nblk = (t_steps + TB - 1) // TB
    ng8 = t_steps // 8

    nc = bacc.Bacc("TRN2", target_bir_lowering=False, debug=False)

    emb = nc.dram_tensor("emb", [V, E], F32, kind="ExternalInput").ap()
    idxd = nc.dram_tensor("idx", [nblk, 128, 4], I32, kind="ExternalInput").ap()
    whd = nc.dram_tensor("wh", [128, NKC * G4], BF16, kind="ExternalInput").ap()
    wid = nc.dram_tensor("wi", [128, NKC * G4], BF16, kind="ExternalInput").ap()
    lend = nc.dram_tensor("lent", [128, 32], F32, kind="ExternalInput").ap()
    biasd = nc.dram_tensor("bias", [128, NMC], F32, kind="ExternalInput").ap()

    outd = nc.dram_tensor("out", [ng8, 128, 256], F32, kind="ExternalOutput").ap()
    debug_xp = os.environ.get("LSTM_DEBUG_XP", "0") == "1"
    if debug_xp:
        xpdbg = nc.dram_tensor("xpdbg", [128, NMC * 512], BF16, kind="ExternalOutput").ap()
    debug_s0 = os.environ.get("LSTM_DEBUG_STEP0", "0") == "1"
    if debug_s0:
        s0dbg = nc.dram_tensor("s0dbg", [6, 128, 64], F32, kind="ExternalOutput").ap()
        pzdbg = nc.dram_tensor("pzdbg", [128, 1536], F32, kind="ExternalOutput").ap()
    fcd = nc.dram_tensor("fc", [128, 32], F32, kind="ExternalOutput").ap()
    fhd = nc.dram_tensor("fh", [128, 32], F32, kind="ExternalOutput").ap()

    with tile.TileContext(nc) as tc:
        const = tc.alloc_tile_pool(name="const", bufs=1)
        psg = tc.alloc_tile_pool(name="psg", bufs=2, space="PSUM")
        psaux = tc.alloc_tile_pool(name="psaux", bufs=2, space="PSUM")
        idxp = tc.alloc_tile_pool(name="idxp", bufs=2)
        xrawp = tc.alloc_tile_pool(name="xrawp", bufs=3)
        xtp = tc.alloc_tile_pool(name="xtp", bufs=2)
        xpp = tc.alloc_tile_pool(name="xpp", bufs=2)
        ewp = tc.alloc_tile_pool(name="ewp", bufs=2)
        stagep = tc.alloc_tile_pool(name="stagep", bufs=3)

        # ---- persistent tiles ----
        wh_sb = const.tile([128, NKC * G4], BF16)
        nc.sync.dma_start(wh_sb[:], whd)
        wi_sb = const.tile([128, NKC * G4], BF16)
        nc.sync.dma_start(wi_sb[:], wid)
        len_sb = const.tile([128, 32], F32)
        nc.sync.dma_start(len_sb[:], lend)
        bias_sb = const.tile([128, NMC], F32)
        nc.sync.dma_start(bias_sb[:], biasd)

        ident = const.tile([128, 128], F32)
        make_identity(nc, ident[:])
        ident_bf = const.tile([128, 128], BF16)
        nc.vector.tensor_copy(ident_bf[:], ident[:])

        c_car = const.tile([128, 32], F32)
        nc.vector.memset(c_car[:], 0.0)
        # h_bf: masked h carry (feeds outputs/final state only).  The matmul
        # rhs uses the UNMASKED h (finished sequences' gate columns are
        # discarded anyway), keeping copy_predicated off the critical path.
        h_bf = const.tile([128, 32], BF16)
        nc.vector.memset(h_bf[:], 0.0)
        h0 = const.tile([128, 32], BF16)
        nc.vector.memset(h0[:], 0.0)

        def make_xproj(blk):
            """Gather + transpose + GEMM for one 64-step block."""
            idx_t = idxp.tile([128, 4], I32, name=f"idx_{blk}", tag="idx")
            nc.sync.dma_start(idx_t[:], idxd[blk])
            xt_sb = xtp.tile([128, NKC * 512], BF16, name=f"xt_{blk}", tag="xt")
            for i in range(4):
                xr = xrawp.tile([128, E], F32, name=f"xr_{blk}_{i}", tag="xr")
                nc.gpsimd.indirect_dma_start(
                    out=xr[:],
                    out_offset=None,
                    in_=emb,
                    in_offset=bass.IndirectOffsetOnAxis(ap=idx_t[:, i : i + 1], axis=0),
                )
                pt = psaux.tile([128, 512], F32, name=f"pt_{blk}_{i}", tag="aux")
                for e in range(4):
                    nc.tensor.transpose(
                        pt[:, e * 128 : (e + 1) * 128],
                        xr[:, e * 128 : (e + 1) * 128],
                        ident[:],
                    )
                # strided copy: psum (e, tok128) -> xt_sb[(e)*512 + i*128 ..]
                src = pt[:].rearrange("p (e q) -> p e q", e=4)
                dst = xt_sb[:].rearrange("p (e w q) -> p e w q", e=4, w=4)[:, :, i, :]
                nc.vector.tensor_copy(dst, src)
            xp_sb = xpp.tile([128, NMC * 512], BF16, name=f"xp_{blk}", tag="xp")
            for mc in range(NMC):
                pg = psaux.tile([128, 512], F32, name=f"pg_{blk}_{mc}", tag="aux")
                for kc in range(NKC):
                    nc.tensor.matmul(
                        pg[:],
                        wi_sb[:, kc * G4 + mc * 128 : kc * G4 + (mc + 1) * 128],
                        xt_sb[:, kc * 512 : (kc + 1) * 512],
                        start=(kc == 0),
                        stop=(kc == NKC - 1),
                    )
                nc.vector.tensor_scalar_add(
                    xp_sb[:, mc * 512 : (mc + 1) * 512], pg[:], bias_sb[:, mc : mc + 1]
                )
            return xp_sb

        # MM issue order: g first, then i, f, o (so tanh(g) can start early)
        mc_order = [8, 9, 10, 11, 0, 1, 2, 3, 4, 5, 6, 7, 12, 13, 14, 15]

        stage = None
        h_prev = h0
        xp_next = make_xproj(0)
        if debug_xp:
            nc.sync.dma_start(xpdbg, xp_next[:])
        for t in range(t_steps):
            blk, t_sub = t // TB, t % TB
            if t_sub == 0:
                xp_sb = xp_next
            # prefetch next block's xproj early so its gather DMA + PE work
            # pipeline behind this block's recurrence steps
            if t_sub == 8 and blk + 1 < nblk:
                xp_next = make_xproj(blk + 1)
            if t % 8 == 0:
                stage = stagep.tile([128, 256], F32, name=f"stage_{t // 8}", tag="st")

            pz = psg.tile([128, 1536], F32, name=f"pz_{t}", tag="pz")
            # inject xproj via identity matmuls, one per gate (N=32, strided
            # rhs over the 4 column-chunks).  start=True clears has_written
            # for the WHOLE bank, so only the first idMM per bank may set it;
            # later idMMs (start=False) overwrite their never-written region.
            xp4 = xp_sb[:].rearrange("p (g c t b) -> p g c t b", g=4, c=4, b=BS)
            for g, goff, st in ((2, 0, True), (0, 512, True), (1, 544, False),
                                (3, 1024, True)):
                nc.tensor.matmul(
                    pz[:, goff : goff + 32],
                    ident_bf[:],
                    xp4[:, g, :, t_sub, :],
                    start=st,
                    stop=False,
                )
            # recurrent part: z += Wh.T @ h
            nsp = int(os.environ.get("LSTM_COLSPLIT", "0"))
            for mc in mc_order:
                off = _mc_off(mc)
                for kc in range(NKC):
                    base = kc * G4 + mc * 128
                    if nsp:
                        w = 128 // nsp
                        for j in range(nsp):
                            nc.tensor.matmul(
                                pz[j * w : (j + 1) * w, off : off + 8],
                                wh_sb[:, base + j * w : base + (j + 1) * w],
                                h_prev[:, kc * 8 : (kc + 1) * 8],
                                start=False,
                                stop=(kc == NKC - 1),
                                tile_position=(0, j * w),
                            )
                    else:
                        nc.tensor.matmul(
                            pz[:, off : off + 8],
                            wh_sb[:, base : base + 128],
                            h_prev[:, kc * 8 : (kc + 1) * 8],
                            start=False,
                            stop=(kc == NKC - 1),
                        )

            # mask for this step (independent of gates; scheduler can hoist)
            msk = ewp.tile([128, 32], mybir.dt.uint8, name=f"msk_{t}", tag="msk")
            nc.vector.tensor_scalar(
                msk[:], len_sb[:], float(t), None, op0=mybir.AluOpType.is_gt
            )

            tg = ewp.tile([128, 32], F32, name=f"tg_{t}", tag="tg")
            nc.scalar.activation(tg[:], pz[:, 0:32], mybir.ActivationFunctionType.Tanh)
            sif = ewp.tile([128, 64], F32, name=f"sif_{t}", tag="sif")
            nc.scalar.activation(
                sif[:], pz[:, 512:576], mybir.ActivationFunctionType.Sigmoid
            )
            so = ewp.tile([128, 32], F32, name=f"so_{t}", tag="so")
            nc.scalar.activation(
                so[:], pz[:, 1024:1056], mybir.ActivationFunctionType.Sigmoid
            )

            t1 = ewp.tile([128, 32], F32, name=f"t1_{t}", tag="t1")
            nc.vector.tensor_mul(t1[:], sif[:, 32:64], c_car[:])
            t2 = ewp.tile([128, 32], F32, name=f"t2_{t}", tag="t2")
            nc.vector.tensor_mul(t2[:], sif[:, 0:32], tg[:])
            cn = ewp.tile([128, 32], F32, name=f"cn_{t}", tag="cn")
            nc.vector.tensor_add(cn[:], t1[:], t2[:])
            tc_t = ewp.tile([128, 32], F32, name=f"tc_{t}", tag="tc")
            nc.scalar.activation(
                tc_t[:], cn[:], mybir.ActivationFunctionType.Tanh
            )
            nc.vector.copy_predicated(c_car[:], msk[:], cn[:])

            # h path stays bf16: shorter critical tail into the next step's
            # matmuls; outputs take the bf16-rounded h (within error budget).
            hn = ewp.tile([128, 32], BF16, name=f"hn_{t}", tag="hn")
            nc.vector.tensor_mul(hn[:], so[:], tc_t[:])
            h_prev = hn
            nc.vector.copy_predicated(h_bf[:], msk[:], hn[:])
            if debug_s0 and t == 0:
                pzsb = ewp.tile([128, 1536], F32, name="pzsb_dbg", tag="pzdbg", bufs=1)
                nc.vector.tensor_copy(pzsb[:], pz[:])
                nc.sync.dma_start(pzdbg, pzsb[:])
                for di, dt_ in enumerate([tg, so, t1, t2, cn, hn]):
                    nc.sync.dma_start(s0dbg[di][:, 0:32], dt_[:])
                nc.sync.dma_start(s0dbg[1][:, 32:64], sif[:, 0:32])
            s8 = t % 8
            nc.scalar.copy(stage[:, s8 * 32 : (s8 + 1) * 32], h_bf[:])
            if s8 == 7:
                nc.sync.dma_start(outd[t // 8], stage[:])

        nc.sync.dma_start(fcd, c_car[:])
        h_f32 = ewp.tile([128, 32], F32, name="h_f32_fin", tag="hf")
        nc.scalar.copy(h_f32[:], h_bf[:])
        nc.sync.dma_start(fhd, h_f32[:])

        for p in (stagep, ewp, xpp, xtp, xrawp, idxp, psaux, psg, const):
            p.release()

    nc.compile()
    return nc


_NC_CACHE = {}


def _get_nc(t_steps):
    if t_steps not in _NC_CACHE:
        _NC_CACHE[t_steps] = build_nc(t_steps)
    return _NC_CACHE[t_steps]


def _prep_weights(Wi, Wh, b):
    """[E, 4H] f32 -> [128, 4*4H] bf16 stationary layout (kc chunks side by side)."""
    def prep(w):
        w4 = w.reshape(NKC, 128, G4).transpose(1, 0, 2).reshape(128, NKC * G4)
        return np.ascontiguousarray(w4).astype(ml_dtypes.bfloat16)

    bias = np.ascontiguousarray(b.reshape(NMC, 128).T).astype(np.float32)
    return prep(Wi), prep(Wh), bias


def _build_in_maps(inp, t_steps):
    inputs = np.asarray(inp["inputs"], dtype=np.int32)
    lengths = np.asarray(inp["lengths"], dtype=np.int32)
    embedding = np.ascontiguousarray(np.asarray(inp["embedding"], dtype=np.float32))
    Wi = np.asarray(inp["Wi"], dtype=np.float32)
    Wh = np.asarray(inp["Wh"], dtype=np.float32)
    b = np.asarray(inp["b"], dtype=np.float32)

    wi_p, wh_p, bias_p = _prep_weights(Wi, Wh, b)

    nblk = (t_steps + TB - 1) // TB
    in_maps = []
    for k in range(NCORES):
        sh = slice(k * BS, (k + 1) * BS)
        toks = inputs[sh, :t_steps]  # [BS, t_steps]
        if toks.shape[1] < nblk * TB:  # dev mode: pad to full gather blocks
            toks = np.pad(toks, ((0, 0), (0, nblk * TB - toks.shape[1])))
        # idx[blk, p, i] = token at (t = blk*64 + (i*128+p)//8, b = (i*128+p)%8)
        jj = np.arange(4)[None, :] * 128 + np.arange(128)[:, None]  # [p, i]
        idx = np.zeros((nblk, 128, 4), dtype=np.int32)
        for blk in range(nblk):
            tt = blk * TB + jj // BS
            bb = jj % BS
            idx[blk] = toks[bb, tt]
        lent = np.broadcast_to(
            np.tile(lengths[sh].astype(np.float32), NKC)[None, :], (128, 32)
        ).copy()
        in_maps.append(
            {
                "emb": embedding,
                "idx": idx,
                "wh": wh_p,
                "wi": wi_p,
                "lent": lent,
                "bias": bias_p,
            }
        )
    return in_maps


def kernel(inputs, lengths, embedding, Wi, Wh, b, t_steps=None):
    t_steps = t_steps or int(os.environ.get("LSTM_T_STEPS", T))
    nc = _get_nc(t_steps)
    in_maps = _build_in_maps(
        {"inputs": inputs, "lengths": lengths, "embedding": embedding,
         "Wi": Wi, "Wh": Wh, "b": b},
        t_steps,
    )
    res = run_bass_kernel_spmd(nc, in_maps, core_ids=list(range(NCORES)))

    outputs = np.zeros((B, T, H), dtype=np.float32)
    c_f = np.zeros((B, H), dtype=np.float32)
    h_f = np.zeros((B, H), dtype=np.float32)
    for k in range(NCORES):
        r = res.results[k]
        # out [ng8, 128, 256]: [g8, p, (s, c, b)] -> [b, t, c*128 + p]
        o = r["out"].reshape(t_steps // 8, 128, 8, 4, BS)
        o = o.transpose(4, 0, 2, 3, 1).reshape(BS, t_steps, H)
        outputs[k * BS : (k + 1) * BS, :t_steps] = o
        # fc/fh [128, 32] = [p, (c, b)] -> [b, c*128 + p]
        fc = r["fc"].reshape(128, NKC, BS).transpose(2, 1, 0).reshape(BS, H)
        fh = r["fh"].reshape(128, NKC, BS).transpose(2, 1, 0).reshape(BS, H)
        c_f[k * BS : (k + 1) * BS] = fc
        h_f[k * BS : (k + 1) * BS] = fh
    if t_steps < T:
        # dev mode: outputs beyond t_steps replicate the frozen value like the
        # reference would only if lengths <= t_steps; leave zeros (test.py
        # compares only the first t_steps in this mode)
        pass
    return outputs, (c_f, h_f)
